# revision 10
# baseline (speedup 1.0000x reference)
"""AutoDiscretizationEmbedding kernel for 8 Trainium2 NeuronCores.

Math (per token t with scalar x_t):
    h      = leaky_relu(x_t * w1 + b1, 0.1)          # [BINS]
    logits = h + h @ w2.T + b2                        # [BINS]
    out_t  = softmax(logits) @ emb                    # [DIM]

Device mapping (all intermediates kept bins-on-partitions, "^T" layout):
  - host folds the +h residual into the w2 matmul via W2' = w2 + I, and
    pre-transposes it into matmul-lhsT layout,
  - h^T = w1b1.T @ [x; 1]  (K=2 matmul; ones row appended host-side),
  - leaky via one DVE scalar_tensor_tensor: max(h, 0.1*h),
  - u^T = exp(logits^T + b2) via one ScalarE activation (per-partition bias),
  - softmax normalizer folded into the embedding matmul: an all-ones
    column is appended to emb host-side, so Z_t = sum_k u[k,t] comes from
    a 1-wide matmul with the same stationary weights; out = (u^T.T @ emb)
    scaled per-token by 1/Z on ScalarE/VectorE.
Sharding: pure data-parallel on the flattened 65536 tokens, 8192 per core.
"""

import numpy as np

B, S = 8, 8192
BINS, DIM = 100, 512
NCORES = 8
NTOK = (B * S) // NCORES          # tokens per core
CHUNK = 512                       # tokens per pipeline chunk
NSUB = CHUNK // 128               # 128-token subtiles per chunk
NCH = NTOK // CHUNK
USE_F32R = False                   # single-pass fp32 matmuls (4x faster than fp32)
LEAKY_MODE = "lrelu"              # "lrelu": 1 ScalarE op; "two_op": ACT scale + DVE max

_CACHE = {}


def _build_nc():
    import concourse.tile as tile
    from concourse import bacc, mybir

    f32 = mybir.dt.float32
    # fp32r = single-pass fp32 matmul dtype; the BIR verifier requires every
    # producer of an fp32r matmul operand to also be declared fp32r.
    fr = mybir.dt.float32r if USE_F32R else f32
    Act = mybir.ActivationFunctionType
    Alu = mybir.AluOpType

    nc = bacc.Bacc("TRN2", target_bir_lowering=False, debug=False,
                   num_devices=NCORES)
    xo_d = nc.dram_tensor("xo", [2, NTOK], fr, kind="ExternalInput").ap()
    w1b1_d = nc.dram_tensor("w1b1", [2, BINS], fr, kind="ExternalInput").ap()
    w2ti_d = nc.dram_tensor("w2ti", [BINS, BINS], fr, kind="ExternalInput").ap()
    b2c_d = nc.dram_tensor("b2c", [BINS, 1], f32, kind="ExternalInput").ap()
    embo_d = nc.dram_tensor("embo", [BINS, DIM + 4], fr, kind="ExternalInput").ap()
    out_d = nc.dram_tensor("out", [NTOK, DIM], f32, kind="ExternalOutput").ap()

    with tile.TileContext(nc) as tc:
        with (
            tc.tile_pool(name="const", bufs=1) as cpool,
            tc.tile_pool(name="hT", bufs=2) as hpool,
            tc.tile_pool(name="uT", bufs=2) as upool,
            tc.tile_pool(name="ost", bufs=3) as opool,
            tc.tile_pool(name="rc", bufs=2) as rpool,
            tc.tile_pool(name="ph", bufs=2, space="PSUM") as ph,
            tc.tile_pool(name="pl", bufs=2, space="PSUM") as pl,
            tc.tile_pool(name="po", bufs=2, space="PSUM") as po,
            tc.tile_pool(name="pz", bufs=2, space="PSUM") as pz,
        ):
            xo = cpool.tile([2, NTOK], fr)
            nc.sync.dma_start(xo[:], xo_d[:])
            w1b1 = cpool.tile([2, BINS], fr)
            nc.sync.dma_start(w1b1[:], w1b1_d[:])
            w2ti = cpool.tile([BINS, BINS], fr)
            nc.sync.dma_start(w2ti[:], w2ti_d[:])
            b2c = cpool.tile([BINS, 1], f32)
            nc.sync.dma_start(b2c[:], b2c_d[:])
            embo = cpool.tile([BINS, DIM + 4], fr)
            nc.sync.dma_start(embo[:], embo_d[:])

            for ch in range(NCH):
                t0 = ch * CHUNK
                h_ps = ph.tile([BINS, CHUNK], f32)
                nc.tensor.matmul(h_ps[:], w1b1[:], xo[:, t0:t0 + CHUNK],
                                 start=True, stop=True)
                hT = hpool.tile([BINS, CHUNK], fr)
                if LEAKY_MODE == "lrelu":
                    nc.scalar.activation(hT[:], h_ps[:], Act.Lrelu, alpha=0.1)
                else:
                    # leaky(x) = max(x, 0.1x); DVE may read only one PSUM input
                    t01 = hpool.tile([BINS, CHUNK], f32, tag="t01")
                    nc.scalar.activation(t01[:], h_ps[:], Act.Copy, scale=0.1)
                    nc.vector.tensor_max(hT[:], h_ps[:], t01[:])
                l_ps = pl.tile([BINS, CHUNK], f32)
                nc.tensor.matmul(l_ps[:], w2ti[:], hT[:],
                                 start=True, stop=True)
                uT = upool.tile([BINS, CHUNK], fr)
                nc.scalar.activation(uT[:], l_ps[:], Act.Exp, bias=b2c[:])

                ost = opool.tile([128, NSUB * DIM], f32)
                for j in range(NSUB):
                    u_j = uT[:, j * 128:(j + 1) * 128]
                    z_ps = pz.tile([128, 4], f32)
                    nc.tensor.matmul(z_ps[:], u_j, embo[:, DIM:DIM + 4],
                                     start=True, stop=True)
                    o_ps = po.tile([128, DIM], f32)
                    nc.tensor.matmul(o_ps[:], u_j, embo[:, 0:DIM],
                                     start=True, stop=True)
                    rc = rpool.tile([128, 1], f32)
                    nc.vector.reciprocal(rc[:], z_ps[:, 0:1])
                    dst = ost[:, j * DIM:(j + 1) * DIM]
                    if j % 2 == 0:
                        nc.scalar.activation(dst, o_ps[:], Act.Copy, scale=rc[:])
                    else:
                        nc.vector.tensor_scalar_mul(dst, o_ps[:], rc[:])

                out_view = out_d[t0:t0 + CHUNK, :].rearrange(
                    "(a p) d -> p a d", p=128)
                nc.sync.dma_start(
                    out_view, ost[:].rearrange("p (a d) -> p a d", d=DIM))
    nc.compile()
    return nc


def _prep_in_maps(x, w1, b1, w2, b2, emb):
    x = np.ascontiguousarray(np.asarray(x, dtype=np.float32)).reshape(B * S)
    w1 = np.asarray(w1, dtype=np.float32)
    b1 = np.asarray(b1, dtype=np.float32)
    w2 = np.asarray(w2, dtype=np.float32)
    b2 = np.asarray(b2, dtype=np.float32)
    emb = np.asarray(emb, dtype=np.float32)

    w1b1 = np.ascontiguousarray(np.stack([w1[:, 0], b1]))            # [2, BINS]
    w2ti = np.ascontiguousarray((w2 + np.eye(BINS, dtype=np.float32)).T)
    b2c = np.ascontiguousarray(b2.reshape(BINS, 1))
    embo = np.ascontiguousarray(
        np.concatenate([emb, np.ones((BINS, 4), np.float32)], axis=1))
    ones = np.ones((1, NTOK), np.float32)

    in_maps = []
    for c in range(NCORES):
        xo = np.ascontiguousarray(
            np.concatenate([x[c * NTOK:(c + 1) * NTOK][None, :], ones], axis=0))
        in_maps.append({"xo": xo, "w1b1": w1b1, "w2ti": w2ti,
                        "b2c": b2c, "embo": embo})
    return in_maps


def _run(in_maps, trace=False, **kw):
    from concourse.bass_utils import run_bass_kernel_spmd
    if "nc" not in _CACHE:
        _CACHE["nc"] = _build_nc()
    return run_bass_kernel_spmd(_CACHE["nc"], in_maps,
                                list(range(NCORES)), trace=trace, **kw)


def kernel(**inputs):
    in_maps = _prep_in_maps(inputs["x"], inputs["w1"], inputs["b1"],
                            inputs["w2"], inputs["b2"], inputs["emb"])
    res = _run(in_maps)
    out = np.stack([res.results[c]["out"] for c in range(NCORES)])
    return out.reshape(B, S, DIM).astype(np.float32, copy=False)
